# revision 30
# baseline (speedup 1.0000x reference)
"""Multi-head attention (B=4, L=2048, D=1024, H=16) on 8 Trainium2 NeuronCores.

Sharding: core c = (batch b = c//2, query-half qh = c%2). Each core computes
all 16 heads for its 1024 query rows against the full 2048 keys/values of its
batch. Fully SPMD, no collectives.

All matmuls bf16 (fp32 PSUM accumulate). Per-core pipeline:
  1. kpT (zero-padded per head half: kpTA rows 0-63 live, kpTB rows 64-127
     live), qpT (W-stationary), vp[L x dh] (x-stationary) projections; q-side
     prescaled by 128*log2(e)/sqrt(dh) on host.
  2. per head-pair, per 512-col query chunk, per 2-j-chunk group:
       S^T = kpT?^T qpT as full-K=128 matmuls (zero rows contribute nothing;
       full-array matmuls keep the PE HAM clock-gate at 8/8).
       P = exp(S^T): scalar-engine Exp (scale=1/A) for most groups; some
       groups use a DVE staggered Schraudolph fast-exp (two int16 bit-packs
       half a mantissa period apart, averaged) to offload the ACT engine.
       outT[dh,i] += vp^T P with a ones column giving Z (softmax denom).
       Z -> 1/Z via DMA-transposed reciprocal; gpsimd partition-broadcast;
       DVE multiply into outT (head B hops partitions via SBUF DMA).
  3. out[l,:] = outT^T woT + bo.
"""

import math
import sys

if "/opt/trn_rl_repo" not in sys.path:
    sys.path.insert(0, "/opt/trn_rl_repo")

import numpy as np

import concourse.bacc as bacc
import concourse.tile as tile
from concourse import mybir
from concourse.bass_utils import run_bass_kernel_spmd

N_CORES = 8
B, L, D = 4, 2048, 1024
NH, DH = 16, 64          # heads, head dim
LQ = L // 2              # query rows per core
F32 = mybir.dt.float32
BF = mybir.dt.bfloat16
I16 = mybir.dt.int16

KC = D // 128            # 8 contraction chunks for projections
NJ = L // 128            # 16 key j-chunks
NI = LQ // 512           # 2 query i-chunks of 512
NPAIR = NH // 2          # 8 head pairs
JGROUPS = [2] * 8

# exp-domain scale folded into w_q on the host: scores arrive in PSUM as
# s * 128*log2(e), ready for both the bit-pack fast-exp and ACT Exp(scale=1/A).
EXPA = 128.0 * math.log2(math.e)
# Staggered-pair Schraudolph constants (DVE f32->i16 convert rounds to
# nearest).  The pair sum t2/sqrt2 + t1 with these biases equals e^s to
# +-1.3% max / 0.47% rms with mean ratio 1.0 — consistent with the exact
# ACT-exp groups it is mixed with inside one softmax row.
_EXPB0 = 128.0 * (127.0 - 0.043035) - 1.8511
EXPB1 = _EXPB0 - 128.0
EXPB2 = _EXPB0 - 64.0
# (group, side) pairs whose exp runs on DVE fast-exp instead of ACT.
EXP_DVE = {(1, 1), (3, 1), (5, 1)}


def build_program(n_cores=N_CORES):
    nc = bacc.Bacc("TRN2", target_bir_lowering=False, debug=False,
                   num_devices=n_cores)
    with tile.TileContext(nc) as tc:
        _emit(nc, tc)
    nc.compile()
    return nc


def _emit(nc, tc):
    from contextlib import ExitStack

    top = ExitStack()
    dram = top.enter_context(tc.tile_pool(name="dram", bufs=1, space="DRAM"))
    xqT = dram.tile([D, LQ], BF, kind="ExternalInput", name="xqT", uniquify=False)
    xkT = dram.tile([D, L], BF, kind="ExternalInput", name="xkT", uniquify=False)
    xvT = dram.tile([D, L], BF, kind="ExternalInput", name="xvT", uniquify=False)
    wqT = dram.tile([D, D], BF, kind="ExternalInput", name="wqT", uniquify=False)
    wkT = dram.tile([D, D], BF, kind="ExternalInput", name="wkT", uniquify=False)
    wvT = dram.tile([D, D], BF, kind="ExternalInput", name="wvT", uniquify=False)
    woT = dram.tile([D, D], BF, kind="ExternalInput", name="woT", uniquify=False)
    bqc = dram.tile([128, KC], F32, kind="ExternalInput", name="bqc", uniquify=False)
    bkc = dram.tile([128, KC], F32, kind="ExternalInput", name="bkc", uniquify=False)
    bvr = dram.tile([1, D], BF, kind="ExternalInput", name="bvr", uniquify=False)
    bor = dram.tile([1, D], BF, kind="ExternalInput", name="bor", uniquify=False)
    c_or = dram.tile([1, 128], BF, kind="ExternalInput", name="c_or", uniquify=False)
    out = dram.tile([LQ, D], F32, kind="ExternalOutput", name="out", uniquify=False)

    # persistent SBUF
    pers = top.enter_context(tc.tile_pool(name="pers", bufs=1))
    # kpTA: head-A dims live in rows 0-63, rows 64-127 zero; kpTB vice versa.
    kpTA = [pers.tile([128, L], BF, name=f"kpTA{m}") for m in range(KC)]
    kpTB = [pers.tile([128, L], BF, name=f"kpTB{m}") for m in range(KC)]
    qpT = [pers.tile([128, LQ], BF, name=f"qpT{m}") for m in range(KC)]
    # vpa: per j-chunk, 16 heads x (64 value cols + 1 ones col) -> Z via PV
    vpa = [pers.tile([128, NH * 65], BF, name=f"vpa{m}") for m in range(NJ)]
    ones1 = pers.tile([1, 128], BF, name="ones1")
    bq_sb = pers.tile([128, KC], F32, name="bq_sb")
    bk_sb = pers.tile([128, KC], F32, name="bk_sb")
    bv_sb = pers.tile([1, D], BF, name="bv_sb")
    bo_sb = pers.tile([1, D], BF, name="bo_sb")

    nc.sync.dma_start(out=ones1[:], in_=c_or[:])
    nc.sync.dma_start(out=bq_sb[:], in_=bqc[:])
    nc.sync.dma_start(out=bk_sb[:], in_=bkc[:])
    nc.sync.dma_start(out=bv_sb[:], in_=bvr[:])
    nc.sync.dma_start(out=bo_sb[:], in_=bor[:])
    for m in range(NJ):
        nc.vector.memset(
            vpa[m].rearrange("p (h c) -> p h c", c=65)[:, :, 64:65], 1.0)
    for m in range(KC):
        nc.gpsimd.memset(kpTA[m][64:128, :], 0.0)
        nc.gpsimd.memset(kpTB[m][0:64, :], 0.0)

    xkT_r = xkT.rearrange("(kc p) l -> p kc l", p=128)
    xqT_r = xqT.rearrange("(kc p) l -> p kc l", p=128)
    xvT_r = xvT.rearrange("(kc p) l -> p kc l", p=128)
    wqT_r = wqT.rearrange("(kc p) m -> p kc m", p=128)
    wkT_r = wkT.rearrange("(kc p) m -> p kc m", p=128)
    wvT_r = wvT.rearrange("(kc p) m -> p kc m", p=128)
    woT_r = woT.rearrange("(kc p) m -> p kc m", p=128)

    # ---- phase 1: projections -------------------------------------------
    with ExitStack() as proj_ctx:
        px = proj_ctx.enter_context(tc.tile_pool(name="px", bufs=1))
        pxb = proj_ctx.enter_context(tc.tile_pool(name="pxb", bufs=2))
        pxv = proj_ctx.enter_context(tc.tile_pool(name="pxv", bufs=3))
        ppq = proj_ctx.enter_context(tc.tile_pool(name="ppq", bufs=2, space="PSUM"))
        ppv = proj_ctx.enter_context(tc.tile_pool(name="ppv", bufs=2, space="PSUM"))

        # warm-up burst: junk matmuls over the (tiny, already-loaded) ones
        # tile keep the PE HAM un-throttled while the big input DMAs land.
        wps = ppv.tile([128, 2, 512], F32, tag="pv")
        for _ in range(28):
            nc.tensor.matmul(wps[:, 0, 0:128], ones1[0:1, :], ones1[0:1, 0:128],
                             start=True, stop=True, skip_group_check=True)

        wq_sb = px.tile([128, KC, D], BF, tag="wq")
        wk_sb = px.tile([128, KC, D], BF, tag="wk")
        wv_sb = px.tile([128, KC, D], BF, tag="wv")
        # x staging: one rotating [128, KC, 1024] buffer pool shared by
        # xq (one buf) and the two L-halves of xk.
        xq_sb = pxb.tile([128, KC, 1024], BF, tag="x")
        xk_h = [pxb.tile([128, KC, 1024], BF, tag="x", name=f"xkh{i}")
                for i in range(2)]
        nc.sync.dma_start(out=xq_sb[:], in_=xqT_r[:])
        nc.sync.dma_start(out=wq_sb[:], in_=wqT_r[:])
        nc.sync.dma_start(out=xk_h[0][:], in_=xkT_r[:, :, 0:1024])
        nc.sync.dma_start(out=wk_sb[:], in_=wkT_r[:])
        nc.sync.dma_start(out=xk_h[1][:], in_=xkT_r[:, :, 1024:2048])
        nc.sync.dma_start(out=wv_sb[:], in_=wvT_r[:])

        # qpT: psum[dh128, l] = sum_kc wqT[:,kc,m128].T @ xqT[:,kc,:]
        for m in range(KC):
            ps = ppq.tile([128, 2, 512], F32, tag="pq")
            for kc in range(KC):
                lw = wq_sb[:, kc, m * 128:(m + 1) * 128]
                for n in range(2):
                    nc.tensor.matmul(ps[:, n, :], lw,
                                     xq_sb[:, kc, n * 512:(n + 1) * 512],
                                     start=(kc == 0), stop=(kc == KC - 1))
            for n in range(2):
                nc.vector.tensor_scalar_add(
                    qpT[m][:, n * 512:(n + 1) * 512], ps[:, n, :],
                    bq_sb[:, m:m + 1])

        # kpT halves into the zero-padded per-head tiles
        for h in range(2):
            for m in range(KC):
                ps = ppq.tile([128, 2, 512], F32, tag="pq")
                for kc in range(KC):
                    lw = wk_sb[:, kc, m * 128:(m + 1) * 128]
                    for n in range(2):
                        nc.tensor.matmul(ps[:, n, :], lw,
                                         xk_h[h][:, kc, n * 512:(n + 1) * 512],
                                         start=(kc == 0), stop=(kc == KC - 1))
                for n in range(2):
                    nsl = slice((2 * h + n) * 512, (2 * h + n + 1) * 512)
                    nc.vector.tensor_scalar_add(
                        kpTA[m][0:64, nsl], ps[0:64, n, :], bk_sb[0:64, m:m + 1])
                    nc.vector.tensor_scalar_add(
                        kpTB[m][64:128, nsl], ps[64:128, n, :],
                        bk_sb[64:128, m:m + 1])

        # vp: psum[l128, dh512] = bias + sum_kc xvT[:,kc,m128].T @ wvT[:,kc,n512]
        for mv in range(NJ):
            xvb = pxv.tile([128, KC, 128], BF, tag="xv")
            nc.sync.dma_start(out=xvb[:], in_=xvT_r[:, :, mv * 128:(mv + 1) * 128])
            ps = ppv.tile([128, 2, 512], F32, tag="pv")
            for n in range(2):
                nc.tensor.matmul(ps[:, n, :], ones1[0:1, :],
                                 bv_sb[0:1, n * 512:(n + 1) * 512],
                                 start=True, stop=False)
            for kc in range(KC):
                lw = xvb[:, kc, :]
                for n in range(2):
                    nc.tensor.matmul(ps[:, n, :], lw,
                                     wv_sb[:, kc, n * 512:(n + 1) * 512],
                                     start=False, stop=(kc == KC - 1))
            vpa_r = vpa[mv].rearrange("p (h c) -> p h c", c=65)
            for n in range(2):
                nc.scalar.copy(
                    vpa_r[:, 8 * n:8 * (n + 1), 0:64],
                    ps[:, n, :].rearrange("p (h c) -> p h c", c=64))

    # ---- phases 2+3 ------------------------------------------------------
    late = top.enter_context(ExitStack())
    pout = late.enter_context(tc.tile_pool(name="pout", bufs=1))
    pwo = late.enter_context(tc.tile_pool(name="pwo", bufs=1))
    outT = [pout.tile([128, LQ], BF, tag=f"outT{m}", name=f"outT{m}")
            for m in range(KC)]
    # stage woT for phase 3 while PE crunches attention
    wo_sb = pwo.tile([128, KC, D], BF, tag="wo")
    nc.sync.dma_start(out=wo_sb[:], in_=woT_r[:])
    with ExitStack() as attn_ctx:
        psS = attn_ctx.enter_context(tc.tile_pool(name="psS", bufs=1, space="PSUM"))
        psO = attn_ctx.enter_context(tc.tile_pool(name="psO", bufs=1, space="PSUM"))
        pP = attn_ctx.enter_context(tc.tile_pool(name="pP", bufs=3))
        pT = attn_ctx.enter_context(tc.tile_pool(name="pT", bufs=2))
        prc = attn_ctx.enter_context(tc.tile_pool(name="prc", bufs=2))

        escale = float(1.0 / EXPA)
        for p in range(NPAIR):
            hA, hB = 2 * p, 2 * p + 1
            for ic in range(NI):
                isl = slice(ic * 512, (ic + 1) * 512)
                # rows 0-63: head output; row 64: Z (from vpa's ones column)
                ozA = psO.tile([65, 512], F32, tag="ozA")
                ozB = psO.tile([65, 512], F32, tag="ozB")

                def emit_scores(g, gs, jbase):
                    sA = psS.tile([128, 1024], F32, tag="sA")
                    # DVE-exp groups park their B scores in a third tag so the
                    # slow (3-op) DVE exp never blocks the next group's scores.
                    btag = "sBd" if (g, 1) in EXP_DVE else "sB"
                    sB = psS.tile([128, 1024], F32, tag=btag)
                    for gg in range(gs):
                        jc = jbase + gg
                        jsl = slice(jc * 128, (jc + 1) * 128)
                        gsl = slice(gg * 512, (gg + 1) * 512)
                        nc.tensor.matmul(sB[:, gsl], kpTB[p][:, jsl],
                                         qpT[p][:, isl])
                        nc.tensor.matmul(sA[:, gsl], kpTA[p][:, jsl],
                                         qpT[p][:, isl])
                    return sA, sB

                def emit_exp(g, gs, sA, sB):
                    w = gs * 512
                    eP = [None, None]
                    for side, s_ps in ((1, sB), (0, sA)):
                        pX = pP.tile([128, 1024], BF, tag=f"p{side}")
                        if (g, side) in EXP_DVE:
                            t1 = pT.tile([128, 1024], BF, tag="t1")
                            t2 = pT.tile([128, 1024], BF, tag="t2")
                            nc.vector.tensor_scalar(
                                t1[:, 0:w].bitcast(I16), s_ps[:, 0:w],
                                EXPB1, None, op0=mybir.AluOpType.add)
                            # round(x+B1)+64 == round(x+B1+64) exactly, so the
                            # staggered twin is an int16 add on t1's bits —
                            # 16-bit SBUF->SBUF (2x DVE mode) instead of a
                            # second 1x-mode fp32 PSUM pass.
                            nc.vector.tensor_scalar(
                                t2[:, 0:w].bitcast(I16), t1[:, 0:w].bitcast(I16),
                                64, None, op0=mybir.AluOpType.add)
                            nc.vector.scalar_tensor_tensor(
                                pX[:, 0:w], t2[:, 0:w], 0.70710678,
                                t1[:, 0:w], mybir.AluOpType.mult,
                                mybir.AluOpType.add)
                        else:
                            nc.scalar.activation(
                                pX[:, 0:w], s_ps[:, 0:w],
                                mybir.ActivationFunctionType.Exp, scale=escale)
                        eP[side] = pX
                    return eP

                def emit_pv(g, gs, jbase, eP):
                    first = (g == 0)
                    for gg in range(gs):
                        jc = jbase + gg
                        last = (jc == NJ - 1)
                        gsl = slice(gg * 512, (gg + 1) * 512)
                        nc.tensor.matmul(ozA[:, :],
                                         vpa[jc][:, hA * 65:(hA + 1) * 65],
                                         eP[0][:, gsl],
                                         start=(first and gg == 0), stop=last)
                        nc.tensor.matmul(ozB[:, :],
                                         vpa[jc][:, hB * 65:(hB + 1) * 65],
                                         eP[1][:, gsl],
                                         start=(first and gg == 0), stop=last)

                # software pipeline: PV for group g is emitted after the
                # scores of group g+1, so the PE has queued work while the
                # exp of group g runs on ACT/DVE.
                jb = [0]
                for g, gs in enumerate(JGROUPS):
                    jb.append(jb[-1] + gs)
                pend = []
                for g, gs in enumerate(JGROUPS):
                    sA, sB = emit_scores(g, gs, jb[g])
                    if len(pend) >= 2:
                        emit_pv(*pend.pop(0))
                    eP = emit_exp(g, gs, sA, sB)
                    pend.append((g, gs, jb[g], eP))
                for item in pend:
                    emit_pv(*item)

                # Copy oz to SBUF immediately (frees the PSUM banks for the
                # next slot's PV), then Z -> 1/Z: DMA-reshape to [8,128] so
                # the reciprocal runs 8 lanes wide, gpsimd partition-broadcast,
                # DVE multiply into outT.
                ozsA = prc.tile([65, 512], F32, tag="ozsA")
                ozsB = prc.tile([65, 512], F32, tag="ozsB")
                nc.vector.tensor_copy(ozsA[:], ozA[:, :])
                nc.vector.tensor_copy(ozsB[:], ozB[:, :])
                zrT = prc.tile([8, 128], F32, tag="zrT")
                nc.sync.dma_start(out=zrT[0:4, :], in_=ozsA[64:65, :])
                nc.sync.dma_start(out=zrT[4:8, :], in_=ozsB[64:65, :])
                zr = prc.tile([8, 128], F32, tag="zr")
                nc.vector.reciprocal(zr[:], zrT[:])
                zbA = prc.tile([1, 512], F32, tag="zbA")
                zbB = prc.tile([1, 512], F32, tag="zbB")
                nc.sync.dma_start(out=zbA[:], in_=zr[0:4, :])
                nc.sync.dma_start(out=zbB[:], in_=zr[4:8, :])
                rsbA = prc.tile([64, 512], F32, tag="rA")
                rsbB = prc.tile([64, 512], F32, tag="rB")
                nc.gpsimd.partition_broadcast(rsbA[:], zbA[:])
                nc.gpsimd.partition_broadcast(rsbB[:], zbB[:])
                nc.vector.tensor_mul(outT[p][0:64, isl], ozsA[0:64, :], rsbA[:])
                stB = prc.tile([64, 512], BF, tag="stB")
                nc.vector.tensor_mul(stB[:], ozsB[0:64, :], rsbB[:])
                nc.sync.dma_start(out=outT[p][64:128, isl], in_=stB[:])

    # ---- phase 3: output projection -------------------------------------
    with ExitStack() as fin_ctx:
        fs = fin_ctx.enter_context(tc.tile_pool(name="fs", bufs=3))
        pf = fin_ctx.enter_context(tc.tile_pool(name="pf", bufs=2, space="PSUM"))

        for mo in range(KC):
            msl = slice(mo * 128, (mo + 1) * 128)
            ps = pf.tile([128, 2, 512], F32, tag="pf")
            for n in range(2):
                nc.tensor.matmul(ps[:, n, :], ones1[0:1, :],
                                 bo_sb[0:1, n * 512:(n + 1) * 512],
                                 start=True, stop=False)
            for kc in range(KC):
                lw = outT[kc][:, msl]
                for n in range(2):
                    nc.tensor.matmul(ps[:, n, :], lw,
                                     wo_sb[:, kc, n * 512:(n + 1) * 512],
                                     start=False, stop=(kc == KC - 1))
            for n in range(2):
                nsl = slice(n * 512, (n + 1) * 512)
                ost = fs.tile([128, 512], F32, tag="fs")
                nc.scalar.copy(ost[:], ps[:, n, :])
                nc.sync.dma_start(out=out[msl, nsl], in_=ost[:])

    late.close()


_NC_CACHE = None


def _get_program():
    global _NC_CACHE
    if _NC_CACHE is None:
        _NC_CACHE = build_program()
    return _NC_CACHE


def prep_in_maps(q, k, v, w_q, b_q, w_k, b_k, w_v, b_v, w_o, b_o):
    import ml_dtypes
    f = np.float32
    bf = ml_dtypes.bfloat16
    q, k, v = (np.asarray(t, f) for t in (q, k, v))
    scale = EXPA / math.sqrt(DH)
    wqT = np.ascontiguousarray((np.asarray(w_q, f) * scale).T).astype(bf)
    wkT = np.ascontiguousarray(np.asarray(w_k, f).T).astype(bf)
    wvT = np.ascontiguousarray(np.asarray(w_v, f).T).astype(bf)
    woT = np.ascontiguousarray(np.asarray(w_o, f).T).astype(bf)
    bqc = np.ascontiguousarray((np.asarray(b_q, f) * scale).reshape(KC, 128).T)
    bkc = np.ascontiguousarray(np.asarray(b_k, f).reshape(KC, 128).T)
    bvr = np.asarray(b_v, f).reshape(1, D).astype(bf)
    bor = np.asarray(b_o, f).reshape(1, D).astype(bf)
    c_or = np.ones((1, 128), bf)
    in_maps = []
    for c in range(N_CORES):
        b, qh = c // 2, c % 2
        kTb = np.ascontiguousarray(k[b].T).astype(bf)
        vTb = np.ascontiguousarray(v[b].T).astype(bf)
        qTb = np.ascontiguousarray(q[b].T[:, qh * LQ:(qh + 1) * LQ]).astype(bf)
        in_maps.append({
            "xqT": qTb, "xkT": kTb, "xvT": vTb,
            "wqT": wqT, "wkT": wkT, "wvT": wvT, "woT": woT,
            "bqc": bqc, "bkc": bkc, "bvr": bvr, "bor": bor,
            "c_or": c_or,
        })
    return in_maps


def run(in_maps, trace=False, **kw):
    nc = _get_program()
    return run_bass_kernel_spmd(nc, in_maps, list(range(N_CORES)),
                                trace=trace, **kw)


def kernel(**inputs):
    in_maps = prep_in_maps(**inputs)
    res = run(in_maps)
    out = np.empty((B, L, D), np.float32)
    for c in range(N_CORES):
        b, qh = c // 2, c % 2
        out[b, qh * LQ:(qh + 1) * LQ, :] = res.results[c]["out"]
    return out


# revision 31
# speedup vs baseline: 1.0153x; 1.0153x over previous
"""Multi-head attention (B=4, L=2048, D=1024, H=16) on 8 Trainium2 NeuronCores.

Sharding: core c = (batch b = c//2, query-half qh = c%2). Each core computes
all 16 heads for its 1024 query rows against the full 2048 keys/values of its
batch. Fully SPMD, no collectives.

All matmuls bf16 (fp32 PSUM accumulate). Per-core pipeline:
  1. kpT (zero-padded per head half: kpTA rows 0-63 live, kpTB rows 64-127
     live), qpT (W-stationary), vp[L x dh] (x-stationary) projections; q-side
     prescaled by 128*log2(e)/sqrt(dh) on host.
  2. per head-pair, per 512-col query chunk, per 2-j-chunk group:
       S^T = kpT?^T qpT as full-K=128 matmuls (zero rows contribute nothing;
       full-array matmuls keep the PE HAM clock-gate at 8/8).
       P = exp(S^T): scalar-engine Exp (scale=1/A) for most groups; some
       groups use a DVE staggered Schraudolph fast-exp (two int16 bit-packs
       half a mantissa period apart, averaged) to offload the ACT engine.
       outT[dh,i] += vp^T P with a ones column giving Z (softmax denom).
       Z -> 1/Z via DMA-transposed reciprocal; gpsimd partition-broadcast;
       DVE multiply into outT (head B hops partitions via SBUF DMA).
  3. out[l,:] = outT^T woT + bo.
"""

import math
import sys

if "/opt/trn_rl_repo" not in sys.path:
    sys.path.insert(0, "/opt/trn_rl_repo")

import numpy as np

import concourse.bacc as bacc
import concourse.tile as tile
from concourse import mybir
from concourse.bass_utils import run_bass_kernel_spmd

N_CORES = 8
B, L, D = 4, 2048, 1024
NH, DH = 16, 64          # heads, head dim
LQ = L // 2              # query rows per core
F32 = mybir.dt.float32
BF = mybir.dt.bfloat16
I16 = mybir.dt.int16

KC = D // 128            # 8 contraction chunks for projections
NJ = L // 128            # 16 key j-chunks
NI = LQ // 512           # 2 query i-chunks of 512
NPAIR = NH // 2          # 8 head pairs
JGROUPS = [2] * 8

# exp-domain scale folded into w_q on the host: scores arrive in PSUM as
# s * 128*log2(e), ready for both the bit-pack fast-exp and ACT Exp(scale=1/A).
EXPA = 128.0 * math.log2(math.e)
# Staggered-pair Schraudolph constants (DVE f32->i16 convert rounds to
# nearest).  The pair sum t2/sqrt2 + t1 with these biases equals e^s to
# +-1.3% max / 0.47% rms with mean ratio 1.0 — consistent with the exact
# ACT-exp groups it is mixed with inside one softmax row.
_EXPB0 = 128.0 * (127.0 - 0.043035) - 1.8511
EXPB1 = _EXPB0 - 128.0
EXPB2 = _EXPB0 - 64.0
# (group, side) pairs whose exp runs on DVE fast-exp instead of ACT.
EXP_DVE = {(1, 1), (4, 1), (7, 1)}


def build_program(n_cores=N_CORES):
    nc = bacc.Bacc("TRN2", target_bir_lowering=False, debug=False,
                   num_devices=n_cores)
    with tile.TileContext(nc) as tc:
        _emit(nc, tc)
    nc.compile()
    return nc


def _emit(nc, tc):
    from contextlib import ExitStack

    top = ExitStack()
    dram = top.enter_context(tc.tile_pool(name="dram", bufs=1, space="DRAM"))
    xqT = dram.tile([D, LQ], BF, kind="ExternalInput", name="xqT", uniquify=False)
    xkT = dram.tile([D, L], BF, kind="ExternalInput", name="xkT", uniquify=False)
    xvT = dram.tile([D, L], BF, kind="ExternalInput", name="xvT", uniquify=False)
    wqT = dram.tile([D, D], BF, kind="ExternalInput", name="wqT", uniquify=False)
    wkT = dram.tile([D, D], BF, kind="ExternalInput", name="wkT", uniquify=False)
    wvT = dram.tile([D, D], BF, kind="ExternalInput", name="wvT", uniquify=False)
    woT = dram.tile([D, D], BF, kind="ExternalInput", name="woT", uniquify=False)
    bqc = dram.tile([128, KC], F32, kind="ExternalInput", name="bqc", uniquify=False)
    bkc = dram.tile([128, KC], F32, kind="ExternalInput", name="bkc", uniquify=False)
    bvr = dram.tile([1, D], BF, kind="ExternalInput", name="bvr", uniquify=False)
    bor = dram.tile([1, D], BF, kind="ExternalInput", name="bor", uniquify=False)
    c_or = dram.tile([1, 128], BF, kind="ExternalInput", name="c_or", uniquify=False)
    out = dram.tile([LQ, D], F32, kind="ExternalOutput", name="out", uniquify=False)

    # persistent SBUF
    pers = top.enter_context(tc.tile_pool(name="pers", bufs=1))
    # kpTA: head-A dims live in rows 0-63, rows 64-127 zero; kpTB vice versa.
    kpTA = [pers.tile([128, L], BF, name=f"kpTA{m}") for m in range(KC)]
    kpTB = [pers.tile([128, L], BF, name=f"kpTB{m}") for m in range(KC)]
    qpT = [pers.tile([128, LQ], BF, name=f"qpT{m}") for m in range(KC)]
    # vpa: per j-chunk, 16 heads x (64 value cols + 1 ones col) -> Z via PV
    vpa = [pers.tile([128, NH * 65], BF, name=f"vpa{m}") for m in range(NJ)]
    ones1 = pers.tile([1, 128], BF, name="ones1")
    bq_sb = pers.tile([128, KC], F32, name="bq_sb")
    bk_sb = pers.tile([128, KC], F32, name="bk_sb")
    bv_sb = pers.tile([1, D], BF, name="bv_sb")
    bo_sb = pers.tile([1, D], BF, name="bo_sb")

    nc.sync.dma_start(out=ones1[:], in_=c_or[:])
    nc.sync.dma_start(out=bq_sb[:], in_=bqc[:])
    nc.sync.dma_start(out=bk_sb[:], in_=bkc[:])
    nc.sync.dma_start(out=bv_sb[:], in_=bvr[:])
    nc.sync.dma_start(out=bo_sb[:], in_=bor[:])
    for m in range(NJ):
        nc.vector.memset(
            vpa[m].rearrange("p (h c) -> p h c", c=65)[:, :, 64:65], 1.0)
    for m in range(KC):
        nc.gpsimd.memset(kpTA[m][64:128, :], 0.0)
        nc.gpsimd.memset(kpTB[m][0:64, :], 0.0)

    xkT_r = xkT.rearrange("(kc p) l -> p kc l", p=128)
    xqT_r = xqT.rearrange("(kc p) l -> p kc l", p=128)
    xvT_r = xvT.rearrange("(kc p) l -> p kc l", p=128)
    wqT_r = wqT.rearrange("(kc p) m -> p kc m", p=128)
    wkT_r = wkT.rearrange("(kc p) m -> p kc m", p=128)
    wvT_r = wvT.rearrange("(kc p) m -> p kc m", p=128)
    woT_r = woT.rearrange("(kc p) m -> p kc m", p=128)

    # ---- phase 1: projections -------------------------------------------
    with ExitStack() as proj_ctx:
        px = proj_ctx.enter_context(tc.tile_pool(name="px", bufs=1))
        pxb = proj_ctx.enter_context(tc.tile_pool(name="pxb", bufs=2))
        pxv = proj_ctx.enter_context(tc.tile_pool(name="pxv", bufs=3))
        ppq = proj_ctx.enter_context(tc.tile_pool(name="ppq", bufs=2, space="PSUM"))
        ppv = proj_ctx.enter_context(tc.tile_pool(name="ppv", bufs=2, space="PSUM"))

        # warm-up burst: junk matmuls over the (tiny, already-loaded) ones
        # tile keep the PE HAM un-throttled while the big input DMAs land.
        wps = ppv.tile([128, 2, 512], F32, tag="pv")
        for _ in range(28):
            nc.tensor.matmul(wps[:, 0, 0:128], ones1[0:1, :], ones1[0:1, 0:128],
                             start=True, stop=True, skip_group_check=True)

        wq_sb = px.tile([128, KC, D], BF, tag="wq")
        wk_sb = px.tile([128, KC, D], BF, tag="wk")
        wv_sb = px.tile([128, KC, D], BF, tag="wv")
        # x staging: one rotating [128, KC, 1024] buffer pool shared by
        # xq (one buf) and the two L-halves of xk.
        xq_sb = pxb.tile([128, KC, 1024], BF, tag="x")
        xk_h = [pxb.tile([128, KC, 1024], BF, tag="x", name=f"xkh{i}")
                for i in range(2)]
        nc.sync.dma_start(out=xq_sb[:], in_=xqT_r[:])
        nc.sync.dma_start(out=wq_sb[:], in_=wqT_r[:])
        nc.sync.dma_start(out=xk_h[0][:], in_=xkT_r[:, :, 0:1024])
        nc.sync.dma_start(out=wk_sb[:], in_=wkT_r[:])
        nc.sync.dma_start(out=xk_h[1][:], in_=xkT_r[:, :, 1024:2048])
        nc.sync.dma_start(out=wv_sb[:], in_=wvT_r[:])

        # qpT: psum[dh128, l] = sum_kc wqT[:,kc,m128].T @ xqT[:,kc,:]
        for m in range(KC):
            ps = ppq.tile([128, 2, 512], F32, tag="pq")
            for kc in range(KC):
                lw = wq_sb[:, kc, m * 128:(m + 1) * 128]
                for n in range(2):
                    nc.tensor.matmul(ps[:, n, :], lw,
                                     xq_sb[:, kc, n * 512:(n + 1) * 512],
                                     start=(kc == 0), stop=(kc == KC - 1))
            for n in range(2):
                nc.vector.tensor_scalar_add(
                    qpT[m][:, n * 512:(n + 1) * 512], ps[:, n, :],
                    bq_sb[:, m:m + 1])

        # kpT halves into the zero-padded per-head tiles
        for h in range(2):
            for m in range(KC):
                ps = ppq.tile([128, 2, 512], F32, tag="pq")
                for kc in range(KC):
                    lw = wk_sb[:, kc, m * 128:(m + 1) * 128]
                    for n in range(2):
                        nc.tensor.matmul(ps[:, n, :], lw,
                                         xk_h[h][:, kc, n * 512:(n + 1) * 512],
                                         start=(kc == 0), stop=(kc == KC - 1))
                for n in range(2):
                    nsl = slice((2 * h + n) * 512, (2 * h + n + 1) * 512)
                    nc.vector.tensor_scalar_add(
                        kpTA[m][0:64, nsl], ps[0:64, n, :], bk_sb[0:64, m:m + 1])
                    nc.vector.tensor_scalar_add(
                        kpTB[m][64:128, nsl], ps[64:128, n, :],
                        bk_sb[64:128, m:m + 1])

        # vp: psum[l128, dh512] = bias + sum_kc xvT[:,kc,m128].T @ wvT[:,kc,n512]
        for mv in range(NJ):
            xvb = pxv.tile([128, KC, 128], BF, tag="xv")
            nc.sync.dma_start(out=xvb[:], in_=xvT_r[:, :, mv * 128:(mv + 1) * 128])
            ps = ppv.tile([128, 2, 512], F32, tag="pv")
            for n in range(2):
                nc.tensor.matmul(ps[:, n, :], ones1[0:1, :],
                                 bv_sb[0:1, n * 512:(n + 1) * 512],
                                 start=True, stop=False)
            for kc in range(KC):
                lw = xvb[:, kc, :]
                for n in range(2):
                    nc.tensor.matmul(ps[:, n, :], lw,
                                     wv_sb[:, kc, n * 512:(n + 1) * 512],
                                     start=False, stop=(kc == KC - 1))
            vpa_r = vpa[mv].rearrange("p (h c) -> p h c", c=65)
            for n in range(2):
                nc.scalar.copy(
                    vpa_r[:, 8 * n:8 * (n + 1), 0:64],
                    ps[:, n, :].rearrange("p (h c) -> p h c", c=64))

    # ---- phases 2+3 ------------------------------------------------------
    late = top.enter_context(ExitStack())
    pout = late.enter_context(tc.tile_pool(name="pout", bufs=1))
    pwo = late.enter_context(tc.tile_pool(name="pwo", bufs=1))
    outT = [pout.tile([128, LQ], BF, tag=f"outT{m}", name=f"outT{m}")
            for m in range(KC)]
    # stage woT for phase 3 while PE crunches attention
    wo_sb = pwo.tile([128, KC, D], BF, tag="wo")
    nc.sync.dma_start(out=wo_sb[:], in_=woT_r[:])
    with ExitStack() as attn_ctx:
        psS = attn_ctx.enter_context(tc.tile_pool(name="psS", bufs=1, space="PSUM"))
        psO = attn_ctx.enter_context(tc.tile_pool(name="psO", bufs=1, space="PSUM"))
        pP = attn_ctx.enter_context(tc.tile_pool(name="pP", bufs=3))
        pT = attn_ctx.enter_context(tc.tile_pool(name="pT", bufs=2))
        prc = attn_ctx.enter_context(tc.tile_pool(name="prc", bufs=2))

        escale = float(1.0 / EXPA)
        for p in range(NPAIR):
            hA, hB = 2 * p, 2 * p + 1
            for ic in range(NI):
                isl = slice(ic * 512, (ic + 1) * 512)
                # rows 0-63: head output; row 64: Z (from vpa's ones column)
                ozA = psO.tile([65, 512], F32, tag="ozA")
                ozB = psO.tile([65, 512], F32, tag="ozB")

                def emit_scores(g, gs, jbase):
                    sA = psS.tile([128, 1024], F32, tag="sA")
                    # DVE-exp groups park their B scores in a third tag so the
                    # slow (3-op) DVE exp never blocks the next group's scores.
                    btag = "sBd" if (g, 1) in EXP_DVE else "sB"
                    sB = psS.tile([128, 1024], F32, tag=btag)
                    for gg in range(gs):
                        jc = jbase + gg
                        jsl = slice(jc * 128, (jc + 1) * 128)
                        gsl = slice(gg * 512, (gg + 1) * 512)
                        nc.tensor.matmul(sB[:, gsl], kpTB[p][:, jsl],
                                         qpT[p][:, isl])
                        nc.tensor.matmul(sA[:, gsl], kpTA[p][:, jsl],
                                         qpT[p][:, isl])
                    return sA, sB

                def emit_exp(g, gs, sA, sB):
                    w = gs * 512
                    eP = [None, None]
                    for side, s_ps in ((1, sB), (0, sA)):
                        pX = pP.tile([128, 1024], BF, tag=f"p{side}")
                        if (g, side) in EXP_DVE:
                            t1 = pT.tile([128, 1024], BF, tag="t1")
                            t2 = pT.tile([128, 1024], BF, tag="t2")
                            nc.vector.tensor_scalar(
                                t1[:, 0:w].bitcast(I16), s_ps[:, 0:w],
                                EXPB1, None, op0=mybir.AluOpType.add)
                            # round(x+B1)+64 == round(x+B1+64) exactly, so the
                            # staggered twin is an int16 add on t1's bits —
                            # 16-bit SBUF->SBUF (2x DVE mode) instead of a
                            # second 1x-mode fp32 PSUM pass.
                            nc.vector.tensor_scalar(
                                t2[:, 0:w].bitcast(I16), t1[:, 0:w].bitcast(I16),
                                64, None, op0=mybir.AluOpType.add)
                            nc.vector.scalar_tensor_tensor(
                                pX[:, 0:w], t2[:, 0:w], 0.70710678,
                                t1[:, 0:w], mybir.AluOpType.mult,
                                mybir.AluOpType.add)
                        else:
                            nc.scalar.activation(
                                pX[:, 0:w], s_ps[:, 0:w],
                                mybir.ActivationFunctionType.Exp, scale=escale)
                        eP[side] = pX
                    return eP

                def emit_pv(g, gs, jbase, eP):
                    first = (g == 0)
                    for gg in range(gs):
                        jc = jbase + gg
                        last = (jc == NJ - 1)
                        gsl = slice(gg * 512, (gg + 1) * 512)
                        nc.tensor.matmul(ozA[:, :],
                                         vpa[jc][:, hA * 65:(hA + 1) * 65],
                                         eP[0][:, gsl],
                                         start=(first and gg == 0), stop=last)
                        nc.tensor.matmul(ozB[:, :],
                                         vpa[jc][:, hB * 65:(hB + 1) * 65],
                                         eP[1][:, gsl],
                                         start=(first and gg == 0), stop=last)

                # software pipeline: PV for group g is emitted after the
                # scores of group g+1, so the PE has queued work while the
                # exp of group g runs on ACT/DVE.
                jb = [0]
                for g, gs in enumerate(JGROUPS):
                    jb.append(jb[-1] + gs)
                pend = []
                for g, gs in enumerate(JGROUPS):
                    sA, sB = emit_scores(g, gs, jb[g])
                    if len(pend) >= 2:
                        emit_pv(*pend.pop(0))
                    eP = emit_exp(g, gs, sA, sB)
                    pend.append((g, gs, jb[g], eP))
                for item in pend:
                    emit_pv(*item)

                # Copy oz to SBUF immediately (frees the PSUM banks for the
                # next slot's PV), then Z -> 1/Z: DMA-reshape to [8,128] so
                # the reciprocal runs 8 lanes wide, gpsimd partition-broadcast,
                # DVE multiply into outT.
                ozsA = prc.tile([65, 512], F32, tag="ozsA")
                ozsB = prc.tile([65, 512], F32, tag="ozsB")
                nc.vector.tensor_copy(ozsA[:], ozA[:, :])
                nc.vector.tensor_copy(ozsB[:], ozB[:, :])
                zrT = prc.tile([8, 128], F32, tag="zrT")
                nc.sync.dma_start(out=zrT[0:4, :], in_=ozsA[64:65, :])
                nc.sync.dma_start(out=zrT[4:8, :], in_=ozsB[64:65, :])
                zr = prc.tile([8, 128], F32, tag="zr")
                nc.vector.reciprocal(zr[:], zrT[:])
                zbA = prc.tile([1, 512], F32, tag="zbA")
                zbB = prc.tile([1, 512], F32, tag="zbB")
                nc.sync.dma_start(out=zbA[:], in_=zr[0:4, :])
                nc.sync.dma_start(out=zbB[:], in_=zr[4:8, :])
                rsbA = prc.tile([64, 512], F32, tag="rA")
                rsbB = prc.tile([64, 512], F32, tag="rB")
                nc.gpsimd.partition_broadcast(rsbA[:], zbA[:])
                nc.gpsimd.partition_broadcast(rsbB[:], zbB[:])
                nc.vector.tensor_mul(outT[p][0:64, isl], ozsA[0:64, :], rsbA[:])
                stB = prc.tile([64, 512], BF, tag="stB")
                nc.vector.tensor_mul(stB[:], ozsB[0:64, :], rsbB[:])
                nc.sync.dma_start(out=outT[p][64:128, isl], in_=stB[:])

    # ---- phase 3: output projection -------------------------------------
    with ExitStack() as fin_ctx:
        fs = fin_ctx.enter_context(tc.tile_pool(name="fs", bufs=3))
        pf = fin_ctx.enter_context(tc.tile_pool(name="pf", bufs=2, space="PSUM"))

        for mo in range(KC):
            msl = slice(mo * 128, (mo + 1) * 128)
            ps = pf.tile([128, 2, 512], F32, tag="pf")
            for n in range(2):
                nc.tensor.matmul(ps[:, n, :], ones1[0:1, :],
                                 bo_sb[0:1, n * 512:(n + 1) * 512],
                                 start=True, stop=False)
            for kc in range(KC):
                lw = outT[kc][:, msl]
                for n in range(2):
                    nc.tensor.matmul(ps[:, n, :], lw,
                                     wo_sb[:, kc, n * 512:(n + 1) * 512],
                                     start=False, stop=(kc == KC - 1))
            for n in range(2):
                nsl = slice(n * 512, (n + 1) * 512)
                ost = fs.tile([128, 512], F32, tag="fs")
                nc.scalar.copy(ost[:], ps[:, n, :])
                nc.sync.dma_start(out=out[msl, nsl], in_=ost[:])

    late.close()


_NC_CACHE = None


def _get_program():
    global _NC_CACHE
    if _NC_CACHE is None:
        _NC_CACHE = build_program()
    return _NC_CACHE


def prep_in_maps(q, k, v, w_q, b_q, w_k, b_k, w_v, b_v, w_o, b_o):
    import ml_dtypes
    f = np.float32
    bf = ml_dtypes.bfloat16
    q, k, v = (np.asarray(t, f) for t in (q, k, v))
    scale = EXPA / math.sqrt(DH)
    wqT = np.ascontiguousarray((np.asarray(w_q, f) * scale).T).astype(bf)
    wkT = np.ascontiguousarray(np.asarray(w_k, f).T).astype(bf)
    wvT = np.ascontiguousarray(np.asarray(w_v, f).T).astype(bf)
    woT = np.ascontiguousarray(np.asarray(w_o, f).T).astype(bf)
    bqc = np.ascontiguousarray((np.asarray(b_q, f) * scale).reshape(KC, 128).T)
    bkc = np.ascontiguousarray(np.asarray(b_k, f).reshape(KC, 128).T)
    bvr = np.asarray(b_v, f).reshape(1, D).astype(bf)
    bor = np.asarray(b_o, f).reshape(1, D).astype(bf)
    c_or = np.ones((1, 128), bf)
    in_maps = []
    for c in range(N_CORES):
        b, qh = c // 2, c % 2
        kTb = np.ascontiguousarray(k[b].T).astype(bf)
        vTb = np.ascontiguousarray(v[b].T).astype(bf)
        qTb = np.ascontiguousarray(q[b].T[:, qh * LQ:(qh + 1) * LQ]).astype(bf)
        in_maps.append({
            "xqT": qTb, "xkT": kTb, "xvT": vTb,
            "wqT": wqT, "wkT": wkT, "wvT": wvT, "woT": woT,
            "bqc": bqc, "bkc": bkc, "bvr": bvr, "bor": bor,
            "c_or": c_or,
        })
    return in_maps


def run(in_maps, trace=False, **kw):
    nc = _get_program()
    return run_bass_kernel_spmd(nc, in_maps, list(range(N_CORES)),
                                trace=trace, **kw)


def kernel(**inputs):
    in_maps = prep_in_maps(**inputs)
    res = run(in_maps)
    out = np.empty((B, L, D), np.float32)
    for c in range(N_CORES):
        b, qh = c // 2, c % 2
        out[b, qh * LQ:(qh + 1) * LQ, :] = res.results[c]["out"]
    return out
